# revision 19
# baseline (speedup 1.0000x reference)
import numpy as np
import ml_dtypes

import concourse.bass as bass
import concourse.bacc as bacc
import concourse.mybir as mybir
from concourse.tile import TileContext
from concourse.bass_utils import run_bass_kernel_spmd

BF16 = ml_dtypes.bfloat16
F32 = np.float32

B, H, W, D, K = 4, 384, 384, 16, 32
NCORES = 8
NPIX_TOT = B * H * W
NPIX = NPIX_TOT // NCORES
P = 128
TC = NPIX // P
TCP = 588
NG = TCP // 7
GW = 7 * 17
WCOLS_PAD = 10016
NBANKS_A = 8
LAB_PAD = 100.0

HT = TC // 2
NW = HT // 8
CR = 48

PUSH_MARGIN = 0.25
PUSH_W = 1.0
PULL_W = 0.1
NCMP = K * (K - 1) / 2.0

CONSUMER_PATTERN = "AADAAAAAAADAAAAAAA"
WT = 16

_built = {}


def _build_launch_a():
    nc = bacc.Bacc("TRN2", target_bir_lowering=False, debug=False)
    bf = mybir.dt.bfloat16
    f32 = mybir.dt.float32

    emb17 = nc.dram_tensor("emb17", [P, WCOLS_PAD], bf, kind="ExternalInput")
    onehotA = nc.dram_tensor("onehotA", [P, TCP * K], bf, kind="ExternalInput")
    outA = nc.dram_tensor("outA", [P, NBANKS_A, 7 * K], f32, kind="ExternalOutput")

    with TileContext(nc) as tc:
        with (
            tc.tile_pool(name="sbuf", bufs=1) as pool,
            tc.tile_pool(name="psum", bufs=1, space="PSUM") as psum_pool,
        ):
            emb_sb = pool.tile([P, WCOLS_PAD], bf)
            onehot = pool.tile([P, TCP, K], bf)

            NCH = 4
            ch = WCOLS_PAD // NCH
            och = TCP // NCH
            for i in range(NCH):
                nc.sync.dma_start(
                    out=emb_sb[:, i * ch : (i + 1) * ch],
                    in_=emb17.ap()[:, i * ch : (i + 1) * ch],
                )
                nc.sync.dma_start(
                    out=onehot[:, i * och : (i + 1) * och, :],
                    in_=onehotA.ap().rearrange("p (t k) -> p t k", k=K)[
                        :, i * och : (i + 1) * och, :
                    ],
                )

            banks = [
                psum_pool.tile([P, 7, K], mybir.dt.float32, name=f"acc{b}")
                for b in range(NBANKS_A)
            ]
            for g in range(NG):
                nc.tensor.matmul(
                    banks[g % NBANKS_A][:],
                    emb_sb[:, GW * g : GW * g + 128],
                    onehot[:, 7 * g : 7 * g + 7, :],
                    start=(g < NBANKS_A),
                    stop=(g >= NG - NBANKS_A),
                )

            evac = pool.tile([P, NBANKS_A, 7 * K], f32)
            for b in range(NBANKS_A):
                eng = nc.scalar if b % 2 == 0 else nc.vector
                if b % 2 == 0:
                    nc.scalar.copy(
                        out=evac[:, b, :],
                        in_=banks[b][:].rearrange("p a b -> p (a b)"),
                    )
                else:
                    nc.vector.tensor_copy(
                        out=evac[:, b, :],
                        in_=banks[b][:].rearrange("p a b -> p (a b)"),
                    )
            nc.sync.dma_start(out=outA.ap(), in_=evac[:])
    nc.compile()
    return nc


def _build_launch_b():
    nc = bacc.Bacc("TRN2", target_bir_lowering=False, debug=False)
    bf = mybir.dt.bfloat16
    f32 = mybir.dt.float32

    wev = nc.dram_tensor("wev", [CR, HT * P], bf, kind="ExternalInput")
    wod = nc.dram_tensor("wod", [CR, HT * P], bf, kind="ExternalInput")
    rtab = nc.dram_tensor("rtab", [CR, 4 * D], bf, kind="ExternalInput")
    pacc_d = nc.dram_tensor("pacc", [P, 4], f32, kind="ExternalOutput")

    AF = mybir.ActivationFunctionType

    with TileContext(nc) as tc:
        with (
            tc.tile_pool(name="sbuf", bufs=1) as pool,
            tc.tile_pool(name="work", bufs=3) as wpool,
            tc.tile_pool(name="psum", bufs=2, space="PSUM") as psum_pool,
        ):
            w_sb = pool.tile([P, HT, P], bf)
            rhs_sb = pool.tile([P, 4 * D], bf)
            dist = pool.tile([P, TC, 4], bf)
            sq = pool.tile([P, TC, 4], bf)
            pacc = pool.tile([P, 4], f32)

            nc.sync.dma_start(
                out=rhs_sb[0:CR, :], in_=rtab.ap()
            )
            nc.sync.dma_start(
                out=rhs_sb[64 : 64 + CR, :], in_=rtab.ap()
            )
            NCH = 6
            tch = HT // NCH
            for i in range(NCH):
                sl = slice(i * tch, (i + 1) * tch)
                nc.sync.dma_start(
                    out=w_sb[0:CR, sl, :],
                    in_=wev.ap().rearrange("r (t m) -> r t m", m=P)[:, sl, :],
                )
                nc.sync.dma_start(
                    out=w_sb[64 : 64 + CR, sl, :],
                    in_=wod.ap().rearrange("r (t m) -> r t m", m=P)[:, sl, :],
                )

            nwaves = HT // WT
            for w in range(nwaves):
                t0 = WT * w
                ps = psum_pool.tile(
                    [P, 2, WT, 4, D], mybir.dt.float32, tag="ps", name=f"ps_{w}"
                )
                for j in range(WT):
                    t = t0 + j
                    nc.tensor.matmul(
                        ps[:, 0, j, :, :].rearrange("p a b -> p (a b)"),
                        w_sb[0:CR, t, :],
                        rhs_sb[0:CR, :],
                        start=True,
                        stop=True,
                    )
                    nc.tensor.matmul(
                        ps[:, 1, j, :, :].rearrange("p a b -> p (a b)"),
                        w_sb[64 : 64 + CR, t, :],
                        rhs_sb[64 : 64 + CR, :],
                        start=True,
                        stop=True,
                    )
                kind = CONSUMER_PATTERN[w % len(CONSUMER_PATTERN)]
                out_ap = dist[:, 2 * t0 : 2 * t0 + 2 * WT, :].rearrange(
                    "p (h t) b -> p h t b", h=2
                )
                with nc.allow_low_precision("dist in bf16; error averages out"):
                    if kind == "D":
                        nc.vector.tensor_reduce(
                            out=out_ap,
                            in_=ps[:],
                            axis=mybir.AxisListType.X,
                            op=mybir.AluOpType.add,
                            apply_absolute_value=True,
                        )
                    else:
                        absd = wpool.tile([P, 2, WT, 4, D], bf, tag="absd")
                        nc.scalar.activation(out=absd[:], in_=ps[:], func=AF.Abs)
                        h1 = wpool.tile([P, 2, WT, 4, D // 2], bf, tag="h1")
                        nc.vector.tensor_tensor(
                            out=h1[:],
                            in0=absd[:, :, :, :, 0 : D // 2],
                            in1=absd[:, :, :, :, D // 2 : D],
                            op=mybir.AluOpType.add,
                        )
                        h2 = wpool.tile([P, 2, WT, 4, D // 4], bf, tag="h2")
                        nc.vector.tensor_tensor(
                            out=h2[:],
                            in0=h1[:, :, :, :, 0 : D // 4],
                            in1=h1[:, :, :, :, D // 4 : D // 2],
                            op=mybir.AluOpType.add,
                        )
                        nc.vector.tensor_reduce(
                            out=out_ap,
                            in_=h2[:],
                            axis=mybir.AxisListType.X,
                            op=mybir.AluOpType.add,
                        )

            nc.vector.tensor_tensor(
                out=sq[:], in0=dist[:], in1=dist[:], op=mybir.AluOpType.mult
            )
            for b in range(4):
                nc.vector.tensor_reduce(
                    out=pacc[:, b : b + 1],
                    in_=sq[:, :, b],
                    axis=mybir.AxisListType.X,
                    op=mybir.AluOpType.add,
                )
            nc.sync.dma_start(out=pacc_d.ap(), in_=pacc[:])
    nc.compile()
    return nc


def _build_merged():
    nc = bacc.Bacc("TRN2", target_bir_lowering=False, debug=False, num_devices=NCORES)
    bf = mybir.dt.bfloat16
    f32 = mybir.dt.float32
    AF = mybir.ActivationFunctionType

    NGM = TC // 8
    embA = nc.dram_tensor("embA", [P, TC * D], bf, kind="ExternalInput")
    onehotA = nc.dram_tensor("onehotA", [P, TC * K], bf, kind="ExternalInput")
    wev = nc.dram_tensor("wev", [CR, HT * P], bf, kind="ExternalInput")
    wod = nc.dram_tensor("wod", [CR, HT * P], bf, kind="ExternalInput")
    itab = nc.dram_tensor("itab", [D, 4 * D], bf, kind="ExternalInput")
    ident = nc.dram_tensor("ident", [D, D], bf, kind="ExternalInput")
    recip = nc.dram_tensor("recip", [1, 4 * K], f32, kind="ExternalInput")
    sg_out = nc.dram_tensor("sg", [D, NCORES * K], f32, kind="ExternalOutput")
    pacc_d = nc.dram_tensor("pacc", [P, 4], f32, kind="ExternalOutput")

    with TileContext(nc) as tc:
        with (
            tc.tile_pool(name="sbuf", bufs=1) as pool,
            tc.tile_pool(name="work", bufs=3) as wpool,
            tc.tile_pool(name="dram", bufs=1, space="DRAM") as dram,
        ):
            embA_sb = pool.tile([P, TC * D], bf)
            ohA_sb = pool.tile([P, TC, K], bf)
            w_sb = pool.tile([P, HT, P], bf)
            rhs_sb = pool.tile([P, 4 * D], bf)
            ident_sb = pool.tile([D, D], bf)
            recip_sb = pool.tile([1, 4 * K], f32)
            dist = pool.tile([P, TC, 4], bf)
            sq = pool.tile([P, TC, 4], bf)
            pacc = pool.tile([P, 4], f32)

            NCH = 6
            ech = (TC * D) // NCH
            och = TC // NCH
            for i in range(NCH):
                nc.sync.dma_start(
                    out=embA_sb[:, i * ech : (i + 1) * ech],
                    in_=embA.ap()[:, i * ech : (i + 1) * ech],
                )
                nc.sync.dma_start(
                    out=ohA_sb[:, i * och : (i + 1) * och, :],
                    in_=onehotA.ap().rearrange("p (t k) -> p t k", k=K)[
                        :, i * och : (i + 1) * och, :
                    ],
                )
            nc.sync.dma_start(out=ident_sb[:], in_=ident.ap())
            nc.sync.dma_start(out=recip_sb[:], in_=recip.ap())
            nc.sync.dma_start(out=rhs_sb[32:48, :], in_=itab.ap())
            nc.sync.dma_start(out=rhs_sb[96:112, :], in_=itab.ap())
            WCH = 6
            tch = HT // WCH
            for i in range(WCH):
                sl = slice(i * tch, (i + 1) * tch)
                nc.sync.dma_start(
                    out=w_sb[0:CR, sl, :],
                    in_=wev.ap().rearrange("r (t m) -> r t m", m=P)[:, sl, :],
                )
                nc.sync.dma_start(
                    out=w_sb[64 : 64 + CR, sl, :],
                    in_=wod.ap().rearrange("r (t m) -> r t m", m=P)[:, sl, :],
                )

            S_sb = pool.tile([D, K], f32)
            hs1 = pool.tile([P, 4, 8, K], f32)
            hs2 = pool.tile([P, 2, 8, K], f32)
            bsum = pool.tile([P, 8, K], f32)
            with tc.tile_pool(name="psa", bufs=1, space="PSUM") as psa:
                ps8 = psa.tile([P, 8, 8, K], mybir.dt.float32)
                for g in range(NGM):
                    nc.tensor.matmul(
                        ps8[:, g % 8, :, :],
                        embA_sb[:, 128 * g : 128 * g + 128],
                        ohA_sb[:, 8 * g : 8 * g + 8, :],
                        start=(g < 8),
                        stop=(g >= NGM - 8),
                    )
                ev4 = pool.tile([P, 4, 8, K], f32)
                nc.scalar.copy(out=ev4[:], in_=ps8[:, 0:4])
                nc.vector.tensor_tensor(
                    out=hs1[:], in0=ps8[:, 4:8], in1=ev4[:],
                    op=mybir.AluOpType.add,
                )
            nc.vector.tensor_tensor(
                out=hs2[:], in0=hs1[:, 0:2], in1=hs1[:, 2:4],
                op=mybir.AluOpType.add,
            )
            nc.vector.tensor_tensor(
                out=bsum[:], in0=hs2[:, 0], in1=hs2[:, 1],
                op=mybir.AluOpType.add,
            )
            diag = pool.tile([D, 8, K], f32)
            for j in range(8):
                nc.sync.dma_start(
                    out=diag[:, j, :], in_=bsum[D * j : D * j + D, j, :]
                )
            nc.vector.tensor_reduce(
                out=S_sb[:],
                in_=diag[:].rearrange("p j k -> p k j"),
                axis=mybir.AxisListType.X,
                op=mybir.AluOpType.add,
            )

            sloc_d = dram.tile([D, K], mybir.dt.float32)
            sg_d = dram.tile([NCORES, D, K], mybir.dt.float32)
            nc.gpsimd.dma_start(sloc_d[:], S_sb[:])
            nc.gpsimd.collective_compute(
                "AllGather",
                mybir.AluOpType.bypass,
                replica_groups=[list(range(NCORES))],
                ins=[sloc_d[:].opt()],
                outs=[sg_d[:].opt()],
            )
            sall = pool.tile([D, NCORES, K], f32)
            nc.sync.dma_start(
                out=sall[:], in_=sg_d[:].rearrange("c d k -> d c k")
            )
            nc.sync.dma_start(
                out=sg_out.ap(),
                in_=sall[:].rearrange("d c k -> d (c k)"),
            )

            s4 = pool.tile([D, 4, K], f32)
            nc.vector.tensor_tensor(
                out=s4[:],
                in0=sall[:, 0:NCORES:2, :],
                in1=sall[:, 1:NCORES:2, :],
                op=mybir.AluOpType.add,
            )
            recipb = pool.tile([D, 4 * K], f32)
            nc.gpsimd.partition_broadcast(recipb[:], recip_sb[:])
            centsb = pool.tile([D, 4 * K], bf)
            with nc.allow_low_precision("cents to bf16 for matmul rhs"):
                nc.vector.tensor_tensor(
                    out=centsb[:],
                    in0=s4[:].rearrange("d b k -> d (b k)"),
                    in1=recipb[:],
                    op=mybir.AluOpType.mult,
                )
            with tc.tile_pool(name="pst", bufs=1, space="PSUM") as pst:
                psT = pst.tile([P, D], bf)
                nc.tensor.transpose(psT[:], centsb[:], ident_sb[:])
                for b in range(4):
                    nc.vector.tensor_copy(
                        out=rhs_sb[0:K, D * b : D * b + D],
                        in_=psT[K * b : K * b + K, :],
                    )
            nc.vector.tensor_copy(out=rhs_sb[64:96, :], in_=rhs_sb[0:K, :])

            with tc.tile_pool(name="psb", bufs=2, space="PSUM") as psum_pool:
                nwaves = HT // WT
                for w in range(nwaves):
                    t0 = WT * w
                    ps = psum_pool.tile(
                        [P, 2, WT, 4, D], mybir.dt.float32, tag="ps", name=f"ps_{w}"
                    )
                    for j in range(WT):
                        t = t0 + j
                        nc.tensor.matmul(
                            ps[:, 0, j, :, :].rearrange("p a b -> p (a b)"),
                            w_sb[0:CR, t, :],
                            rhs_sb[0:CR, :],
                            start=True,
                            stop=True,
                        )
                        nc.tensor.matmul(
                            ps[:, 1, j, :, :].rearrange("p a b -> p (a b)"),
                            w_sb[64 : 64 + CR, t, :],
                            rhs_sb[64 : 64 + CR, :],
                            start=True,
                            stop=True,
                        )
                    kind = CONSUMER_PATTERN[w % len(CONSUMER_PATTERN)]
                    out_ap = dist[:, 2 * t0 : 2 * t0 + 2 * WT, :].rearrange(
                        "p (h t) b -> p h t b", h=2
                    )
                    with nc.allow_low_precision("dist bf16; error averages out"):
                        if kind == "D":
                            nc.vector.tensor_reduce(
                                out=out_ap,
                                in_=ps[:],
                                axis=mybir.AxisListType.X,
                                op=mybir.AluOpType.add,
                                apply_absolute_value=True,
                            )
                        else:
                            absd = wpool.tile([P, 2, WT, 4, D], bf, tag="absd")
                            nc.scalar.activation(out=absd[:], in_=ps[:], func=AF.Abs)
                            h1 = wpool.tile([P, 2, WT, 4, D // 2], bf, tag="h1")
                            nc.vector.tensor_tensor(
                                out=h1[:],
                                in0=absd[:, :, :, :, 0 : D // 2],
                                in1=absd[:, :, :, :, D // 2 : D],
                                op=mybir.AluOpType.add,
                            )
                            h2 = wpool.tile([P, 2, WT, 4, D // 4], bf, tag="h2")
                            nc.vector.tensor_tensor(
                                out=h2[:],
                                in0=h1[:, :, :, :, 0 : D // 4],
                                in1=h1[:, :, :, :, D // 4 : D // 2],
                                op=mybir.AluOpType.add,
                            )
                            nc.vector.tensor_reduce(
                                out=out_ap,
                                in_=h2[:],
                                axis=mybir.AxisListType.X,
                                op=mybir.AluOpType.add,
                            )

            nc.vector.tensor_tensor(
                out=sq[:], in0=dist[:], in1=dist[:], op=mybir.AluOpType.mult
            )
            for b in range(4):
                nc.vector.tensor_reduce(
                    out=pacc[:, b : b + 1],
                    in_=sq[:, :, b],
                    axis=mybir.AxisListType.X,
                    op=mybir.AluOpType.add,
                )
            nc.sync.dma_start(out=pacc_d.ap(), in_=pacc[:])
    nc.compile()
    return nc


def _get(name):
    if name not in _built:
        if name == "A":
            _built[name] = _build_launch_a()
        elif name == "M":
            _built[name] = _build_merged()
        else:
            _built[name] = _build_launch_b()
    return _built[name]


def _prep_a(emb_flat, lab_flat):
    in_maps = []
    kk = np.arange(K, dtype=np.int32)
    for c in range(NCORES):
        e = emb_flat[c * NPIX : (c + 1) * NPIX].astype(BF16).reshape(P, TC, D)
        l = lab_flat[c * NPIX : (c + 1) * NPIX].reshape(P, TC)
        e17 = np.zeros((P, TCP, 17), dtype=BF16)
        e17[:, :TC, :D] = e
        e17[:, :, D] = BF16(1.0)
        w = np.zeros((P, WCOLS_PAD), dtype=BF16)
        w[:, : TCP * 17] = e17.reshape(P, TCP * 17)
        oh = np.zeros((P, TCP, K), dtype=BF16)
        oh[:, :TC, :] = (l[:, :, None] == kk[None, None, :]).astype(BF16)
        in_maps.append({"emb17": w, "onehotA": oh.reshape(P, TCP * K)})
    return in_maps


def _reduce_a(results):
    sums = np.zeros((B, K, D), dtype=np.float64)
    cnts = np.zeros((B, K), dtype=np.float64)
    for c in range(NCORES):
        o = results[c]["outA"].astype(np.float64).reshape(P, NBANKS_A, 7, K)
        o = o.sum(axis=1)
        s = c // 2
        for j in range(7):
            blk = o[17 * j : 17 * j + 17, j, :]
            sums[s] += blk[:D].T
            cnts[s] += blk[D]
    cents = sums / np.maximum(cnts, 1.0)[:, :, None]
    cents = np.where(cnts[:, :, None] > 0, cents, 0.0)
    return cents, cnts


def _prep_b(emb_flat, lab_flat, cents):
    cb = cents.astype(F32)
    rtab = np.zeros((CR, 4 * D), dtype=BF16)
    rtab[:K, :] = cb.transpose(1, 0, 2).reshape(K, 4 * D).astype(BF16)
    eye = -np.eye(D, dtype=F32)
    for b in range(4):
        rtab[K:, b * D : (b + 1) * D] = eye.astype(BF16)

    in_maps = []
    kk = np.arange(K, dtype=np.int32)
    for c in range(NCORES):
        e = emb_flat[c * NPIX : (c + 1) * NPIX].astype(BF16).reshape(P, TC, D)
        l = lab_flat[c * NPIX : (c + 1) * NPIX].reshape(P, TC)
        oh = (l.T[:, None, :] == kk[None, :, None]).astype(BF16)
        eT = np.ascontiguousarray(e.transpose(1, 2, 0))
        w_all = np.concatenate([oh, eT], axis=1)
        wev = np.ascontiguousarray(w_all[:HT].transpose(1, 0, 2)).reshape(CR, HT * P)
        wod = np.ascontiguousarray(w_all[HT:].transpose(1, 0, 2)).reshape(CR, HT * P)
        in_maps.append({"wev": wev, "wod": wod, "rtab": rtab.copy()})
    return in_maps


def _prep_m(emb_flat, lab_flat):
    itab = np.zeros((D, 4 * D), dtype=BF16)
    eye = -np.eye(D, dtype=F32)
    for b in range(4):
        itab[:, b * D : (b + 1) * D] = eye.astype(BF16)
    ident = np.eye(D, dtype=F32).astype(BF16)

    cnts = np.zeros((4, K), dtype=np.int64)
    spl = NPIX_TOT // 4
    for b in range(4):
        cnts[b] = np.bincount(lab_flat[b * spl : (b + 1) * spl], minlength=K)
    recip = (1.0 / np.maximum(cnts, 1)).astype(F32).reshape(1, 4 * K)

    in_maps = []
    kk = np.arange(K, dtype=np.int32)
    for c in range(NCORES):
        e = emb_flat[c * NPIX : (c + 1) * NPIX].astype(BF16).reshape(P, TC, D)
        l = lab_flat[c * NPIX : (c + 1) * NPIX].reshape(P, TC)
        oh = (l[:, :, None] == kk[None, None, :]).astype(BF16)
        ohT = (l.T[:, None, :] == kk[None, :, None]).astype(BF16)
        eT = np.ascontiguousarray(e.transpose(1, 2, 0))
        w_all = np.concatenate([ohT, eT], axis=1)
        wev = np.ascontiguousarray(w_all[:HT].transpose(1, 0, 2)).reshape(CR, HT * P)
        wod = np.ascontiguousarray(w_all[HT:].transpose(1, 0, 2)).reshape(CR, HT * P)
        in_maps.append(
            {
                "embA": e.reshape(P, TC * D),
                "onehotA": oh.reshape(P, TC * K),
                "wev": wev,
                "wod": wod,
                "itab": itab.copy(),
                "ident": ident.copy(),
                "recip": recip.copy(),
            }
        )
    return in_maps, cnts


def run_merged(embeddings, labels, trace=False, trace_kwargs=None):
    emb_flat = np.ascontiguousarray(np.asarray(embeddings), dtype=F32).reshape(
        NPIX_TOT, D
    )
    lab_flat = np.ascontiguousarray(np.asarray(labels), dtype=np.int32).reshape(
        NPIX_TOT
    )
    in_maps, cnts = _prep_m(emb_flat, lab_flat)
    kwA = dict(trace=trace, **(trace_kwargs or {}))
    resM = run_bass_kernel_spmd(_get("M"), in_maps, list(range(NCORES)), **kwA)

    sg = resM.results[0]["sg"].astype(np.float64).reshape(D, NCORES, K)
    sums = np.zeros((4, K, D), dtype=np.float64)
    for b in range(4):
        sums[b] = (sg[:, 2 * b, :] + sg[:, 2 * b + 1, :]).T
    cents = sums / np.maximum(cnts, 1)[:, :, None]
    cents = np.where(cnts[:, :, None] > 0, cents, 0.0)
    push = _push_host(cents)

    pull = np.zeros(4, dtype=np.float64)
    for c in range(NCORES):
        pull += resM.results[c]["pacc"].astype(np.float64).sum(axis=0)
    pull /= NPIX_TOT

    loss = np.mean(PUSH_W * push + PULL_W * pull)
    return np.array(loss, dtype=F32), resM


def _push_host(cents):
    cb = cents.astype(np.float64)
    d = np.abs(cb[:, :, None, :] - cb[:, None, :, :]).sum(axis=-1)
    m = np.maximum(PUSH_MARGIN - d, 0.0)
    iu = np.triu(np.ones((K, K), dtype=bool), k=1)
    return (m * m * iu[None]).sum(axis=(1, 2)) / NCMP


def run_launches(embeddings, labels, trace=False, trace_kwargs=None):
    emb_flat = np.ascontiguousarray(np.asarray(embeddings), dtype=F32).reshape(
        NPIX_TOT, D
    )
    lab_flat = np.ascontiguousarray(np.asarray(labels), dtype=np.int32).reshape(
        NPIX_TOT
    )
    core_ids = list(range(NCORES))

    kwA = dict(trace=trace, **(trace_kwargs or {}))
    resA = run_bass_kernel_spmd(_get("A"), _prep_a(emb_flat, lab_flat), core_ids, **kwA)
    cents, _ = _reduce_a(resA.results)

    resB = run_bass_kernel_spmd(
        _get("B"), _prep_b(emb_flat, lab_flat, cents), core_ids, **kwA
    )
    pull = np.zeros(4, dtype=np.float64)
    for c in range(NCORES):
        pull += resB.results[c]["pacc"].astype(np.float64).sum(axis=0)
    pull /= NPIX_TOT

    push = _push_host(cents)

    loss = np.mean(PUSH_W * push + PULL_W * pull)
    return np.array(loss, dtype=F32), resA, resB


def kernel(embeddings, labels):
    loss, _ = run_merged(embeddings, labels, trace=False)
    return loss
